# revision 1
# baseline (speedup 1.0000x reference)
"""AttentionReadout Trainium2 kernel (8-core data-parallel over the graph axis).

Reference computation (per graph of 64 nodes, D=512, H=8 heads, hd=64):
    qkv = x @ in_proj_w.T + in_proj_b ; q,k,v = split(qkv)
    attn = softmax(q k^T / sqrt(hd)) v          (per head)
    attn_out = attn @ out_proj_w.T + out_proj_b
    gates = sigmoid(attn_out @ gate_w.T + gate_b)
    out[g] = sum_n attn_out[n] * gates[n]

Layout strategy (per core: 128 graphs = 8192 nodes):
  - X^T tiles ([d,n], bf16) made via cast-DMA to HBM + DMA-xbar transpose.
  - Q^T,K^T projected directly in [e,n] orientation; V in natural [n,e]
    orientation with a ones column appended (so the ctx matmul also yields
    the softmax denominator).
  - Scores computed transposed, S^T[m,n], per (block of 128 nodes, head);
    exp on ScalarE reads only the two valid 64x64 diagonal quadrants.
  - ctx via K=64 quadrant matmuls (tile_position packed), never touching
    cross-graph garbage.  ctx normalized by 1/rowsum on VectorE, then
    PE-transposed for the out projection in natural [n,e] orientation.
  - gate column folded into the out projection via w_eff = out_proj_w.T@gw;
    sigmoid computed as 0.5*tanh(x/2)+0.5 (tanh shares the exp ACT table).
  - readout as [e,2]-per-block matmuls accumulated transposed, one final
    PE transpose at the end.
"""

import numpy as np
import ml_dtypes

import concourse.bass as bass
import concourse.mybir as mybir
import concourse.tile as tile
from concourse import bacc
from concourse.bass_utils import run_bass_kernel_spmd
from concourse.masks import make_identity

F32 = mybir.dt.float32
BF16 = mybir.dt.bfloat16

N_CORES = 8
D = 512
H = 8
HD = 64
NPG = 64            # nodes per graph
TOTAL = 65536
ROWS = TOTAL // N_CORES      # 8192 nodes per core
GC = ROWS // NPG             # 128 graphs per core
BLK = 128                    # nodes per block (2 graphs)
SBN = 512                    # nodes per superblock (4 blocks, 8 graphs)
NSB = ROWS // SBN            # 16 superblocks
NBLK = SBN // BLK            # 4 blocks per superblock
DC = D // 128                # 4 d-chunks

# module-level switch used by test.py; harness default is no tracing
TRACE = False

try:
    import jax as _jax
    _jax.config.update("jax_compilation_cache_dir", "/tmp/jax_neff_cache")
    _jax.config.update("jax_persistent_cache_min_compile_time_secs", 10)
    _jax.config.update("jax_persistent_cache_min_entry_size_bytes", 0)
except Exception:
    pass


def _build(has_bqk, has_bv, has_bo, has_gb, rows=ROWS, variant=()):
    variant = set(variant)
    stage = 8
    for _v in variant:
        if _v.startswith("s") and _v[1:].isdigit():
            stage = int(_v[1:])
    nsb = rows // SBN
    gc = rows // NPG
    nc = bacc.Bacc(None, target_bir_lowering=False, debug=False)

    xbf = nc.dram_tensor("xbf", [rows, D], BF16, kind="ExternalInput")
    wqk = nc.dram_tensor("wqk", [128, DC, 2 * D], BF16, kind="ExternalInput")
    wv = nc.dram_tensor("wv", [128, DC, D], BF16, kind="ExternalInput")
    wo = nc.dram_tensor("wo", [128, DC, D], BF16, kind="ExternalInput")
    weff = nc.dram_tensor("weff", [128, DC, 1], BF16, kind="ExternalInput")
    if has_bqk:
        bqk = nc.dram_tensor("bqk", [128, 2 * DC], F32, kind="ExternalInput")
    if has_bv:
        bv = nc.dram_tensor("bv", [1, D], F32, kind="ExternalInput")
    if has_bo:
        bo = nc.dram_tensor("bo", [1, D], F32, kind="ExternalInput")
    if has_gb:
        gbh = nc.dram_tensor("gbh", [1, 1], F32, kind="ExternalInput")
    out = nc.dram_tensor("out", [gc, D], F32, kind="ExternalOutput")

    from contextlib import ExitStack
    with tile.TileContext(nc) as tc, ExitStack() as st:
        consts = st.enter_context(tc.tile_pool(name="consts", bufs=1))
        p_xt = st.enter_context(tc.tile_pool(name="p_xt", bufs=3))
        p_qkt = st.enter_context(tc.tile_pool(name="p_qkt", bufs=2))
        p_v = st.enter_context(tc.tile_pool(name="p_v", bufs=3))
        p_attn = st.enter_context(tc.tile_pool(name="p_attn", bufs=3))
        p_ctx = st.enter_context(tc.tile_pool(name="p_ctx", bufs=3))
        p_ctxt = st.enter_context(tc.tile_pool(name="p_ctxt", bufs=3))
        p_ao = st.enter_context(tc.tile_pool(name="p_ao", bufs=6))
        p_small = st.enter_context(tc.tile_pool(name="p_small", bufs=4))
        ps_s = st.enter_context(tc.tile_pool(name="ps_s", bufs=2, space="PSUM"))
        ps_c = st.enter_context(tc.tile_pool(name="ps_c", bufs=2, space="PSUM"))
        ps_b1 = st.enter_context(tc.tile_pool(name="ps_b1", bufs=2, space="PSUM"))
        ps_misc = st.enter_context(tc.tile_pool(name="ps_misc", bufs=2, space="PSUM"))

        # ---- constants / weights ----
        ident_bf = consts.tile([128, 128], BF16, tag="ident_bf")
        make_identity(nc, ident_bf[:])
        ident_f32 = consts.tile([128, 128], F32, tag="ident_f32")
        make_identity(nc, ident_f32[:])

        wqk_sb = consts.tile([128, DC, 2 * D], BF16, tag="wqk")
        nc.sync.dma_start(wqk_sb[:], wqk[:, :, :])
        wv_sb = consts.tile([128, DC, D], BF16, tag="wv")
        nc.sync.dma_start(wv_sb[:], wv[:, :, :])
        wo_sb = consts.tile([128, DC, D], BF16, tag="wo")
        nc.sync.dma_start(wo_sb[:], wo[:, :, :])
        weff_sb = consts.tile([128, DC, 1], BF16, tag="weff")
        nc.sync.dma_start(weff_sb[:], weff[:, :, :])

        if has_bqk:
            bqk_sb = consts.tile([128, 2 * DC], F32, tag="bqk")
            nc.sync.dma_start(bqk_sb[:], bqk[:, :])
        if has_bv:
            bv_row = consts.tile([1, D], F32, tag="bv_row")
            nc.sync.dma_start(bv_row[:], bv[:, :])
            bv_full = consts.tile([128, D], F32, tag="bv_full")
            nc.gpsimd.partition_broadcast(bv_full[:], bv_row[:])
        if has_bo:
            bo_row = consts.tile([1, D], F32, tag="bo_row")
            nc.sync.dma_start(bo_row[:], bo[:, :])
            bo_full = consts.tile([128, D], F32, tag="bo_full")
            nc.gpsimd.partition_broadcast(bo_full[:], bo_row[:])
        if has_gb:
            gbh_row = consts.tile([1, 1], F32, tag="gbh_row")
            nc.sync.dma_start(gbh_row[:], gbh[:, :])
            gbh_full = consts.tile([128, 1], F32, tag="gbh_full")
            nc.gpsimd.partition_broadcast(gbh_full[:], gbh_row[:])

        # transposed final output accumulator [e_part, dc, g]
        outT = consts.tile([128, DC, gc], F32, tag="outT")

        # ---- phase B: per superblock ----
        _reps = 2 if "x2" in variant else 1
        for sb in list(range(nsb)) * _reps:
            r0 = sb * SBN

            # X^T tiles [d_part, dc, n]
            xt = p_xt.tile([128, DC, SBN], BF16, tag="xt")
            if "nodmat" in variant:
                for b in range(NBLK):
                    xnat = p_small.tile([128, D], BF16, tag="xnat")
                    nc.sync.dma_start(
                        xnat[:], xbf[r0 + b * 128:r0 + (b + 1) * 128, :])
                    psx = ps_b1.tile([128, DC, 128], BF16, tag="b1")
                    for dc in range(DC):
                        nc.tensor.transpose(
                            psx[:, dc, :], xnat[:, dc * 128:(dc + 1) * 128],
                            ident_bf[:])
                    nc.vector.tensor_copy(
                        xt[:, :, b * 128:(b + 1) * 128], psx[:])
            else:
                for dc in range(DC):
                    nc.sync.dma_start_transpose(
                        xt[:, dc, :], xbf[r0:r0 + SBN, dc * 128:(dc + 1) * 128])

            if stage <= 1:
                continue
            # Q^T,K^T projection: [e, n] orientation, 8 e-chunks of 128
            qkt = p_qkt.tile([128, 8, SBN], BF16, tag="qkt")
            for ec in range(8):
                ps = ps_b1.tile([128, SBN], F32, tag="b1")
                for dc in range(DC):
                    nc.tensor.matmul(
                        ps[:],
                        wqk_sb[:, dc, ec * 128:(ec + 1) * 128],
                        xt[:, dc, :],
                        start=(dc == 0), stop=(dc == DC - 1))
                if has_bqk:
                    eng = nc.vector if ec % 2 == 0 else nc.scalar
                    if ec % 2 == 0:
                        nc.vector.tensor_scalar_add(
                            qkt[:, ec, :], ps[:], bqk_sb[:, ec:ec + 1])
                    else:
                        nc.scalar.activation(
                            qkt[:, ec, :], ps[:],
                            mybir.ActivationFunctionType.Identity,
                            bias=bqk_sb[:, ec:ec + 1])
                else:
                    if ec % 2 == 0:
                        nc.vector.tensor_copy(qkt[:, ec, :], ps[:])
                    else:
                        nc.scalar.copy(qkt[:, ec, :], ps[:])

            # odd heads' Q^T/K^T rows live at partitions 64:127, which
            # matmuls cannot address as operands (base-64 stationary operand
            # fails at runtime) -- realign them to partitions 0:63 once.
            qko = p_qkt.tile([64, 8, SBN], BF16, tag="qko")
            nc.sync.dma_start(qko[:, :, :], qkt[64:128, :, :])

            if stage <= 2:
                continue
            # V projection (natural [n, e]) per block, with ones column
            v_sbs = []
            for b in range(NBLK):
                ps = ps_b1.tile([128, D], F32, tag="b1")
                for dc in range(DC):
                    nc.tensor.matmul(
                        ps[:],
                        xt[:, dc, b * 128:(b + 1) * 128],
                        wv_sb[:, dc, :],
                        start=(dc == 0), stop=(dc == DC - 1))
                vt = p_v.tile([128, H, HD + 1], BF16, tag="v")
                pv = ps[:].rearrange("p (h c) -> p h c", h=H)
                if has_bv:
                    nc.vector.tensor_tensor(
                        vt[:, :, 0:HD], pv,
                        bv_full[:].rearrange("p (h c) -> p h c", h=H),
                        mybir.AluOpType.add)
                else:
                    if b % 2 == 0:
                        nc.vector.tensor_copy(vt[:, :, 0:HD], pv)
                    else:
                        nc.scalar.copy(vt[:, :, 0:HD], pv)
                nc.vector.memset(vt[:, :, HD:HD + 1], 1.0)
                v_sbs.append(vt)

            if stage <= 3:
                continue
            # per block: attention + out projection + readout
            ao_sbs = []
            psg = ps_misc.tile([128, 64], F32, tag="misc")  # gate cols 32:36, outT 0:32
            for b in range(NBLK):
                n0 = b * 128
                vt = v_sbs[b]
                attn = p_attn.tile([128, H, 128], BF16, tag="attn")
                nc.gpsimd.memset(attn[0:64, :, 64:128], 0.0)
                nc.gpsimd.memset(attn[64:128, :, 0:64], 0.0)
                for hh in range(2):
                    # scores S^T[m, n] for 4 heads
                    pss = ps_s.tile([128, 4, 128], F32, tag="s")
                    for j in range(4):
                        h = hh * 4 + j
                        src_t = qkt if h % 2 == 0 else qko
                        ec_k = 4 + h // 2
                        ec_q = h // 2
                        nc.tensor.matmul(
                            pss[:, j, :],
                            src_t[0:64, ec_k, n0:n0 + 128],
                            src_t[0:64, ec_q, n0:n0 + 128],
                            start=True, stop=True)
                    # exp of the two valid quadrants (scale 1/sqrt(hd))
                    expf = (mybir.ActivationFunctionType.Copy
                            if "noexp" in variant else
                            mybir.ActivationFunctionType.Exp)
                    if "fullexp" in variant:
                        nc.scalar.activation(
                            attn[:, hh * 4:hh * 4 + 4, :],
                            pss[:, :, :],
                            expf, scale=0.125)
                    else:
                        nc.scalar.activation(
                            attn[0:64, hh * 4:hh * 4 + 4, 0:64],
                            pss[0:64, :, 0:64],
                            expf, scale=0.125)
                        nc.scalar.activation(
                            attn[64:128, hh * 4:hh * 4 + 4, 64:128],
                            pss[64:128, :, 64:128],
                            expf, scale=0.125)
                    if stage <= 4:
                        continue
                    # ctx (+rowsum) via quadrant matmuls
                    psc_full = ps_c.tile([128, 512], F32, tag="c")
                    psc = psc_full[:, 0:4 * (HD + 1)].rearrange(
                        "p (h c) -> p h c", c=HD + 1)
                    for j in range(4):
                        h = hh * 4 + j
                        nc.tensor.matmul(
                            psc[:, j, :],
                            attn[:, h, :],
                            vt[:, h, :],
                            start=True, stop=True)
                    rr = p_small.tile([128, 4], F32, tag="rr")
                    nc.vector.reciprocal(rr[:], psc[:, :, HD])
                    if hh == 0:
                        ctx = p_ctx.tile([128, H, HD], BF16, tag="ctx")
                    nc.vector.tensor_tensor(
                        ctx[:, hh * 4:hh * 4 + 4, :],
                        psc[:, :, 0:HD],
                        rr[:, :, None].to_broadcast((128, 4, HD)),
                        mybir.AluOpType.mult)

                if stage <= 5:
                    continue
                # ctx^T via PE transposes
                pst = ps_b1.tile([128, D], BF16, tag="b1")
                cflat = ctx[:].rearrange("p h c -> p (h c)")
                for ec in range(DC):
                    nc.tensor.transpose(
                        pst[:, ec * 128:(ec + 1) * 128],
                        cflat[:, ec * 128:(ec + 1) * 128],
                        ident_bf[:])
                ctxt = p_ctxt.tile([128, DC, 128], BF16, tag="ctxt")
                if b % 2 == 0:
                    nc.vector.tensor_copy(
                        ctxt[:].rearrange("p d n -> p (d n)"), pst[:])
                else:
                    nc.scalar.copy(
                        ctxt[:].rearrange("p d n -> p (d n)"), pst[:])

                if stage <= 6:
                    continue
                # out projection (natural [n, e]) + gate column
                pso = ps_b1.tile([128, D], F32, tag="b1")
                for dc in range(DC):
                    nc.tensor.matmul(
                        pso[:], ctxt[:, dc, :], wo_sb[:, dc, :],
                        start=(dc == 0), stop=(dc == DC - 1))
                for dc in range(DC):
                    nc.tensor.matmul(
                        psg[:, 32 + b:33 + b], ctxt[:, dc, :], weff_sb[:, dc, :],
                        start=(dc == 0), stop=(dc == DC - 1))
                ao = p_ao.tile([128, D], BF16, tag="ao")
                if has_bo:
                    nc.vector.tensor_tensor(
                        ao[:], pso[:], bo_full[:], mybir.AluOpType.add)
                else:
                    nc.scalar.copy(ao[:], pso[:])
                ao_sbs.append(ao)

            if stage <= 7:
                continue
            # gates for the whole superblock
            tsb = p_small.tile([128, 4], F32, tag="tsb")
            tanhf = (mybir.ActivationFunctionType.Copy
                     if "noexp" in variant else
                     mybir.ActivationFunctionType.Tanh)
            if "noexp" in variant:
                nc.scalar.activation(tsb[:], psg[:, 32:36], tanhf, scale=0.5)
            else:
                nc.scalar.activation(
                    tsb[:], psg[:, 32:36], tanhf,
                    bias=(gbh_full[:] if has_gb else 0.0), scale=0.5)
            gsb = p_small.tile([128, 4], BF16, tag="gsb")
            nc.vector.tensor_scalar(
                gsb[:], tsb[:], 0.5, 0.5,
                mybir.AluOpType.mult, mybir.AluOpType.add)
            G = p_small.tile([128, 8], BF16, tag="G")
            nc.vector.memset(G[:], 0.0)
            Gv = G[:].rearrange("p (a t) -> p a t", t=2)
            nc.vector.tensor_copy(Gv[0:64, :, 0], gsb[0:64, :])
            nc.vector.tensor_copy(Gv[64:128, :, 1], gsb[64:128, :])

            # readout: out^T[e, g] accumulated per (block, e-chunk)
            rv = psg[:, 0:32].rearrange("p (e g) -> p e g", e=DC)
            for b in range(NBLK):
                ao = ao_sbs[b]
                for ec in range(DC):
                    nc.tensor.matmul(
                        rv[:, ec, 2 * b:2 * b + 2],
                        ao[:, ec * 128:(ec + 1) * 128],
                        G[:, 2 * b:2 * b + 2],
                        start=True, stop=True)
            nc.vector.tensor_copy(outT[:, :, sb * 8:(sb + 1) * 8], rv)

        # ---- phase C: final transpose of outT -> out [g, e] ----
        if stage >= 8:
            psf = ps_b1.tile([128, D], F32, tag="b1", name="psf")
        if stage >= 8:
            for dc in range(DC):
                nc.tensor.transpose(
                    psf[0:gc, dc * 128:(dc + 1) * 128], outT[:, dc, :],
                    ident_f32[:])
            out_sb = p_small.tile([128, D], F32, tag="osb")
            nc.vector.tensor_copy(out_sb[0:gc, :], psf[0:gc, :])
            nc.sync.dma_start(out[:, :], out_sb[0:gc, :])

    import time as _time
    _t = _time.time()
    nc.compile()
    print(f"[kernel] bacc compile: {_time.time()-_t:.1f}s", flush=True)
    return nc


def kernel(x, batch, in_proj_w, in_proj_b, out_proj_w, out_proj_b,
           gate_w, gate_b):
    x = np.ascontiguousarray(np.asarray(x, dtype=np.float32))
    in_proj_w = np.asarray(in_proj_w, dtype=np.float32)
    in_proj_b = np.asarray(in_proj_b, dtype=np.float32)
    out_proj_w = np.asarray(out_proj_w, dtype=np.float32)
    out_proj_b = np.asarray(out_proj_b, dtype=np.float32)
    gate_w = np.asarray(gate_w, dtype=np.float32)
    gate_b = np.asarray(gate_b, dtype=np.float32)

    # host-side weight prep
    wqkT = in_proj_w[:2 * D].T                              # [512, 1024]
    wqk_h = np.ascontiguousarray(
        wqkT.reshape(DC, 128, 2 * D).transpose(1, 0, 2)).astype(ml_dtypes.bfloat16)
    wvT = in_proj_w[2 * D:].T                               # [512, 512]
    wv_h = np.ascontiguousarray(
        wvT.reshape(DC, 128, D).transpose(1, 0, 2)).astype(ml_dtypes.bfloat16)
    woT = out_proj_w.T                                      # [512, 512]
    wo_h = np.ascontiguousarray(
        woT.reshape(DC, 128, D).transpose(1, 0, 2)).astype(ml_dtypes.bfloat16)
    weff = (out_proj_w.T @ gate_w[0]).astype(np.float32)    # [512]
    weff_h = np.ascontiguousarray(
        weff.reshape(DC, 128, 1).transpose(1, 0, 2)).astype(ml_dtypes.bfloat16)

    bqk_np = in_proj_b[:2 * D]
    bv_np = in_proj_b[2 * D:]
    gb_eff = float(gate_b[0] + out_proj_b @ gate_w[0])
    has_bqk = bool(np.any(bqk_np))
    has_bv = bool(np.any(bv_np))
    has_bo = bool(np.any(out_proj_b))
    has_gb = gb_eff != 0.0

    import time as _time
    _t = _time.time()
    nc = _build(has_bqk, has_bv, has_bo, has_gb)
    print(f"[kernel] build total: {_time.time()-_t:.1f}s", flush=True)

    in_maps = []
    for c in range(N_CORES):
        m = {
            "xbf": np.ascontiguousarray(
                x[c * ROWS:(c + 1) * ROWS]).astype(ml_dtypes.bfloat16),
            "wqk": wqk_h, "wv": wv_h, "wo": wo_h, "weff": weff_h,
        }
        if has_bqk:
            m["bqk"] = np.ascontiguousarray(
                bqk_np.reshape(2 * DC, 128).T).astype(np.float32)
        if has_bv:
            m["bv"] = bv_np.reshape(1, D).astype(np.float32)
        if has_bo:
            m["bo"] = out_proj_b.reshape(1, D).astype(np.float32)
        if has_gb:
            m["gbh"] = np.array([[0.5 * gb_eff]], dtype=np.float32)
        in_maps.append(m)

    kernel.last_nc = nc
    kernel.last_in_maps = in_maps
    kernel.last_flags = (has_bqk, has_bv, has_bo, has_gb)

    res = run_bass_kernel_spmd(
        nc, in_maps, core_ids=list(range(N_CORES)), trace=TRACE)
    if TRACE:
        kernel.last_exec_time_ns = res.exec_time_ns
        kernel.last_results = res

    return np.concatenate([r["out"] for r in res.results], axis=0)


kernel.last_exec_time_ns = None
kernel.last_results = None
kernel.last_nc = None
kernel.last_in_maps = None


def _make_runner(nc, in_maps):
    """Build a repeat-callable PJRT runner for `nc` with device-resident
    inputs (mirrors bass2jax.run_bass_via_pjrt's multi-core path, minus
    output donation so buffers can be reused across timing iterations)."""
    import jax
    from jax.sharding import Mesh, PartitionSpec, NamedSharding
    from jax.experimental.shard_map import shard_map
    from concourse import bass2jax

    bass2jax.install_neuronx_cc_hook()
    n_cores = len(in_maps)

    partition_name = (
        nc.partition_id_tensor.name if nc.partition_id_tensor else None)
    in_names, out_names, out_avals, zero_outs = [], [], [], []
    for alloc in nc.m.functions[0].allocations:
        if not isinstance(alloc, mybir.MemoryLocationSet):
            continue
        name = alloc.memorylocations[0].name
        if alloc.kind == "ExternalInput":
            if name != partition_name:
                in_names.append(name)
        elif alloc.kind == "ExternalOutput":
            shape = tuple(alloc.tensor_shape)
            dtype = mybir.dt.np(alloc.dtype)
            out_avals.append(jax.core.ShapedArray(shape, dtype))
            out_names.append(name)
            zero_outs.append(np.zeros(shape, dtype))
    n_params = len(in_names)
    all_in_names = in_names + out_names
    if partition_name is not None:
        all_in_names = all_in_names + [partition_name]

    def _body(*args):
        operands = list(args)
        if partition_name is not None:
            operands.append(bass2jax.partition_id_tensor())
        outs = bass2jax._bass_exec_p.bind(
            *operands,
            out_avals=tuple(out_avals),
            in_names=tuple(all_in_names),
            out_names=tuple(out_names),
            lowering_input_output_aliases=(),
            sim_require_finite=True,
            sim_require_nnan=True,
            nc=nc,
        )
        return tuple(outs)

    devices = jax.devices()[:n_cores]
    mesh = Mesh(np.asarray(devices), ("core",))
    nsp = len(in_names) + len(out_names)
    sharded = jax.jit(
        shard_map(_body, mesh=mesh,
                  in_specs=(PartitionSpec("core"),) * nsp,
                  out_specs=(PartitionSpec("core"),) * len(out_names),
                  check_rep=False),
        keep_unused=True,
    )
    sharding = NamedSharding(mesh, PartitionSpec("core"))
    concat_in = [
        np.concatenate([np.asarray(in_maps[c][name]) for c in range(n_cores)], axis=0)
        for name in in_names
    ] + [np.zeros((n_cores * z.shape[0], *z.shape[1:]), z.dtype) for z in zero_outs]
    dev_in = [jax.device_put(a, sharding) for a in concat_in]

    def run_once(block=True):
        outs = sharded(*dev_in)
        if block:
            jax.block_until_ready(outs)
        return outs

    return run_once


def _build_empty():
    """Tiny program for measuring per-dispatch overhead."""
    nc = bacc.Bacc(None, target_bir_lowering=False, debug=False)
    a = nc.dram_tensor("a", [128, 128], F32, kind="ExternalInput")
    o = nc.dram_tensor("o", [128, 128], F32, kind="ExternalOutput")
    with tile.TileContext(nc) as tc:
        with tc.tile_pool(name="sb", bufs=1) as sb:
            t = sb.tile([128, 128], F32, tag="t")
            nc.sync.dma_start(t[:], a[:])
            nc.sync.dma_start(o[:, :], t[:])
    nc.compile()
    return nc


def bench(iters=20, warmup=3):
    """Timing of the last-built kernel.

    The axon dispatch path quantizes blocking-call wall time (~75 ms), so a
    single execution cannot be resolved directly.  Instead build a variant
    of the same program with the whole per-superblock pipeline repeated
    twice ("x2") and report min(T_x2) - min(T_x1): the marginal cost of one
    full compute pass, with all dispatch overhead cancelled.
    """
    import time
    assert kernel.last_nc is not None, "call kernel() first"

    runner = _make_runner(kernel.last_nc, kernel.last_in_maps)
    nc_x2 = _build(*kernel.last_flags, variant=("x2",))
    runner_x2 = _make_runner(nc_x2, kernel.last_in_maps)

    def measure(run):
        for _ in range(warmup):
            run()
        ts = []
        for _ in range(iters):
            t0 = time.perf_counter()
            run()
            ts.append(time.perf_counter() - t0)
        ts.sort()
        return ts

    ts_k = measure(runner)
    ts_2 = measure(runner_x2)
    print(f"[bench] x1 min/p25/med: "
          f"{ts_k[0]*1e6:.0f}/{ts_k[len(ts_k)//4]*1e6:.0f}/"
          f"{ts_k[len(ts_k)//2]*1e6:.0f} us")
    print(f"[bench] x2 min/p25/med: "
          f"{ts_2[0]*1e6:.0f}/{ts_2[len(ts_2)//4]*1e6:.0f}/"
          f"{ts_2[len(ts_2)//2]*1e6:.0f} us")
    exec_ns = (ts_2[0] - ts_k[0]) * 1e9
    return exec_ns, ts_k[0] * 1e9, ts_2[0] * 1e9



# revision 2
# speedup vs baseline: 3.5756x; 3.5756x over previous
"""AttentionReadout Trainium2 kernel (8-core data-parallel over the graph axis).

Reference computation (per graph of 64 nodes, D=512, H=8 heads, hd=64):
    qkv = x @ in_proj_w.T + in_proj_b ; q,k,v = split(qkv)
    attn = softmax(q k^T / sqrt(hd)) v          (per head)
    attn_out = attn @ out_proj_w.T + out_proj_b
    gates = sigmoid(attn_out @ gate_w.T + gate_b)
    out[g] = sum_n attn_out[n] * gates[n]

Layout strategy (per core: 128 graphs = 8192 nodes):
  - X^T tiles ([d,n], bf16) made via cast-DMA to HBM + DMA-xbar transpose.
  - Q^T,K^T projected directly in [e,n] orientation; V in natural [n,e]
    orientation with a ones column appended (so the ctx matmul also yields
    the softmax denominator).
  - Scores computed transposed, S^T[m,n], per (block of 128 nodes, head);
    exp on ScalarE reads only the two valid 64x64 diagonal quadrants.
  - ctx via K=64 quadrant matmuls (tile_position packed), never touching
    cross-graph garbage.  ctx normalized by 1/rowsum on VectorE, then
    PE-transposed for the out projection in natural [n,e] orientation.
  - gate column folded into the out projection via w_eff = out_proj_w.T@gw;
    sigmoid computed as 0.5*tanh(x/2)+0.5 (tanh shares the exp ACT table).
  - readout as [e,2]-per-block matmuls accumulated transposed, one final
    PE transpose at the end.
"""

import numpy as np
import ml_dtypes

import concourse.bass as bass
import concourse.mybir as mybir
import concourse.tile as tile
from concourse import bacc
from concourse.bass_utils import run_bass_kernel_spmd
from concourse.masks import make_identity

F32 = mybir.dt.float32
BF16 = mybir.dt.bfloat16

N_CORES = 8
D = 512
H = 8
HD = 64
NPG = 64            # nodes per graph
TOTAL = 65536
ROWS = TOTAL // N_CORES      # 8192 nodes per core
GC = ROWS // NPG             # 128 graphs per core
BLK = 128                    # nodes per block (2 graphs)
SBN = 512                    # nodes per superblock (4 blocks, 8 graphs)
NSB = ROWS // SBN            # 16 superblocks
NBLK = SBN // BLK            # 4 blocks per superblock
DC = D // 128                # 4 d-chunks

# module-level switch used by test.py; harness default is no tracing
TRACE = False

try:
    import jax as _jax
    _jax.config.update("jax_compilation_cache_dir", "/tmp/jax_neff_cache")
    _jax.config.update("jax_persistent_cache_min_compile_time_secs", 10)
    _jax.config.update("jax_persistent_cache_min_entry_size_bytes", 0)
except Exception:
    pass


def _build(has_bqk, has_bv, has_bo, has_gb, rows=ROWS, variant=()):
    variant = set(variant)
    stage = 8
    for _v in variant:
        if _v.startswith("s") and _v[1:].isdigit():
            stage = int(_v[1:])
    nsb = rows // SBN
    gc = rows // NPG
    nc = bacc.Bacc(None, target_bir_lowering=False, debug=False)

    xbf = nc.dram_tensor("xbf", [rows, D], BF16, kind="ExternalInput")
    wqk = nc.dram_tensor("wqk", [128, DC, 2 * D], BF16, kind="ExternalInput")
    wv = nc.dram_tensor("wv", [128, DC, D], BF16, kind="ExternalInput")
    wo = nc.dram_tensor("wo", [128, DC, D], BF16, kind="ExternalInput")
    weff = nc.dram_tensor("weff", [128, DC, 1], BF16, kind="ExternalInput")
    if has_bqk:
        bqk = nc.dram_tensor("bqk", [128, 2 * DC], F32, kind="ExternalInput")
    if has_bv:
        bv = nc.dram_tensor("bv", [1, D], F32, kind="ExternalInput")
    if has_bo:
        bo = nc.dram_tensor("bo", [1, D], F32, kind="ExternalInput")
    if has_gb:
        gbh = nc.dram_tensor("gbh", [1, 1], F32, kind="ExternalInput")
    out = nc.dram_tensor("out", [gc, D], F32, kind="ExternalOutput")

    from contextlib import ExitStack
    with tile.TileContext(nc) as tc, ExitStack() as st:
        consts = st.enter_context(tc.tile_pool(name="consts", bufs=1))
        p_xt = st.enter_context(tc.tile_pool(name="p_xt", bufs=3))
        p_qkt = st.enter_context(tc.tile_pool(name="p_qkt", bufs=2))
        p_v = st.enter_context(tc.tile_pool(name="p_v", bufs=3))
        p_attn = st.enter_context(tc.tile_pool(name="p_attn", bufs=3))
        p_ctx = st.enter_context(tc.tile_pool(name="p_ctx", bufs=3))
        p_ctxt = st.enter_context(tc.tile_pool(name="p_ctxt", bufs=3))
        p_ao = st.enter_context(tc.tile_pool(name="p_ao", bufs=6))
        p_small = st.enter_context(tc.tile_pool(name="p_small", bufs=4))
        ps_s = st.enter_context(tc.tile_pool(name="ps_s", bufs=2, space="PSUM"))
        ps_c = st.enter_context(tc.tile_pool(name="ps_c", bufs=2, space="PSUM"))
        ps_b1 = st.enter_context(tc.tile_pool(name="ps_b1", bufs=2, space="PSUM"))
        ps_misc = st.enter_context(tc.tile_pool(name="ps_misc", bufs=2, space="PSUM"))

        # ---- constants / weights ----
        ident_bf = consts.tile([128, 128], BF16, tag="ident_bf")
        make_identity(nc, ident_bf[:])
        ident_f32 = consts.tile([128, 128], F32, tag="ident_f32")
        make_identity(nc, ident_f32[:])

        wqk_sb = consts.tile([128, DC, 2 * D], BF16, tag="wqk")
        nc.sync.dma_start(wqk_sb[:], wqk[:, :, :])
        wv_sb = consts.tile([128, DC, D], BF16, tag="wv")
        nc.sync.dma_start(wv_sb[:], wv[:, :, :])
        wo_sb = consts.tile([128, DC, D], BF16, tag="wo")
        nc.sync.dma_start(wo_sb[:], wo[:, :, :])
        weff_sb = consts.tile([128, DC, 1], BF16, tag="weff")
        nc.sync.dma_start(weff_sb[:], weff[:, :, :])

        if has_bqk:
            bqk_sb = consts.tile([128, 2 * DC], F32, tag="bqk")
            nc.sync.dma_start(bqk_sb[:], bqk[:, :])
        if has_bv:
            bv_row = consts.tile([1, D], F32, tag="bv_row")
            nc.sync.dma_start(bv_row[:], bv[:, :])
            bv_full = consts.tile([128, D], F32, tag="bv_full")
            nc.gpsimd.partition_broadcast(bv_full[:], bv_row[:])
        if has_bo:
            bo_row = consts.tile([1, D], F32, tag="bo_row")
            nc.sync.dma_start(bo_row[:], bo[:, :])
            bo_full = consts.tile([128, D], F32, tag="bo_full")
            nc.gpsimd.partition_broadcast(bo_full[:], bo_row[:])
        if has_gb:
            gbh_row = consts.tile([1, 1], F32, tag="gbh_row")
            nc.sync.dma_start(gbh_row[:], gbh[:, :])
            gbh_full = consts.tile([128, 1], F32, tag="gbh_full")
            nc.gpsimd.partition_broadcast(gbh_full[:], gbh_row[:])

        # transposed final output accumulator [e_part, dc, g]
        outT = consts.tile([128, DC, gc], F32, tag="outT")

        # ---- phase B: per superblock ----
        _reps = 1
        for _v in variant:
            if _v.startswith("x") and _v[1:].isdigit():
                _reps = int(_v[1:])
        for sb in list(range(nsb)) * _reps:
            r0 = sb * SBN

            # X^T tiles [d_part, dc, n]
            xt = p_xt.tile([128, DC, SBN], BF16, tag="xt")
            if "nodmat" in variant:
                for b in range(NBLK):
                    xnat = p_small.tile([128, D], BF16, tag="xnat")
                    nc.sync.dma_start(
                        xnat[:], xbf[r0 + b * 128:r0 + (b + 1) * 128, :])
                    psx = ps_b1.tile([128, DC, 128], BF16, tag="b1")
                    for dc in range(DC):
                        nc.tensor.transpose(
                            psx[:, dc, :], xnat[:, dc * 128:(dc + 1) * 128],
                            ident_bf[:])
                    nc.vector.tensor_copy(
                        xt[:, :, b * 128:(b + 1) * 128], psx[:])
            else:
                for dc in range(DC):
                    nc.sync.dma_start_transpose(
                        xt[:, dc, :], xbf[r0:r0 + SBN, dc * 128:(dc + 1) * 128])

            if stage <= 1:
                continue
            # Q^T,K^T projection: [e, n] orientation, 8 e-chunks of 128
            qkt = p_qkt.tile([128, 8, SBN], BF16, tag="qkt")
            for ec in range(8):
                ps = ps_b1.tile([128, SBN], F32, tag="b1")
                for dc in range(DC):
                    nc.tensor.matmul(
                        ps[:],
                        wqk_sb[:, dc, ec * 128:(ec + 1) * 128],
                        xt[:, dc, :],
                        start=(dc == 0), stop=(dc == DC - 1))
                if has_bqk:
                    eng = nc.vector if ec % 2 == 0 else nc.scalar
                    if ec % 2 == 0:
                        nc.vector.tensor_scalar_add(
                            qkt[:, ec, :], ps[:], bqk_sb[:, ec:ec + 1])
                    else:
                        nc.scalar.activation(
                            qkt[:, ec, :], ps[:],
                            mybir.ActivationFunctionType.Identity,
                            bias=bqk_sb[:, ec:ec + 1])
                else:
                    if ec % 2 == 0:
                        nc.vector.tensor_copy(qkt[:, ec, :], ps[:])
                    else:
                        nc.scalar.copy(qkt[:, ec, :], ps[:])

            # odd heads' Q^T/K^T rows live at partitions 64:127, which
            # matmuls cannot address as operands (base-64 stationary operand
            # fails at runtime) -- realign them to partitions 0:63 once.
            qko = p_qkt.tile([64, 8, SBN], BF16, tag="qko")
            nc.sync.dma_start(qko[:, :, :], qkt[64:128, :, :])

            if stage <= 2:
                continue
            # V projection (natural [n, e]) per block, with ones column
            v_sbs = []
            for b in range(NBLK):
                ps = ps_b1.tile([128, D], F32, tag="b1")
                for dc in range(DC):
                    nc.tensor.matmul(
                        ps[:],
                        xt[:, dc, b * 128:(b + 1) * 128],
                        wv_sb[:, dc, :],
                        start=(dc == 0), stop=(dc == DC - 1))
                vt = p_v.tile([128, H, HD + 1], BF16, tag="v")
                pv = ps[:].rearrange("p (h c) -> p h c", h=H)
                if has_bv:
                    nc.vector.tensor_tensor(
                        vt[:, :, 0:HD], pv,
                        bv_full[:].rearrange("p (h c) -> p h c", h=H),
                        mybir.AluOpType.add)
                else:
                    if b % 2 == 0:
                        nc.vector.tensor_copy(vt[:, :, 0:HD], pv)
                    else:
                        nc.scalar.copy(vt[:, :, 0:HD], pv)
                nc.vector.memset(vt[:, :, HD:HD + 1], 1.0)
                v_sbs.append(vt)

            if stage <= 3:
                continue
            # per block: attention + out projection + readout
            ao_sbs = []
            psg = ps_misc.tile([128, 64], F32, tag="misc")  # gate cols 32:36, outT 0:32
            for b in range(NBLK):
                n0 = b * 128
                vt = v_sbs[b]
                attn = p_attn.tile([128, H, 128], BF16, tag="attn")
                nc.gpsimd.memset(attn[0:64, :, 64:128], 0.0)
                nc.gpsimd.memset(attn[64:128, :, 0:64], 0.0)
                for hh in range(2):
                    # scores S^T[m, n] for 4 heads
                    pss = ps_s.tile([128, 4, 128], F32, tag="s")
                    for j in range(4):
                        h = hh * 4 + j
                        src_t = qkt if h % 2 == 0 else qko
                        ec_k = 4 + h // 2
                        ec_q = h // 2
                        nc.tensor.matmul(
                            pss[:, j, :],
                            src_t[0:64, ec_k, n0:n0 + 128],
                            src_t[0:64, ec_q, n0:n0 + 128],
                            start=True, stop=True)
                    # exp of the two valid quadrants (scale 1/sqrt(hd))
                    expf = (mybir.ActivationFunctionType.Copy
                            if "noexp" in variant else
                            mybir.ActivationFunctionType.Exp)
                    if "fullexp" in variant:
                        nc.scalar.activation(
                            attn[:, hh * 4:hh * 4 + 4, :],
                            pss[:, :, :],
                            expf, scale=0.125)
                    else:
                        nc.scalar.activation(
                            attn[0:64, hh * 4:hh * 4 + 4, 0:64],
                            pss[0:64, :, 0:64],
                            expf, scale=0.125)
                        nc.scalar.activation(
                            attn[64:128, hh * 4:hh * 4 + 4, 64:128],
                            pss[64:128, :, 64:128],
                            expf, scale=0.125)
                    if stage <= 4:
                        continue
                    # ctx (+rowsum) via quadrant matmuls
                    psc_full = ps_c.tile([128, 512], F32, tag="c")
                    psc = psc_full[:, 0:4 * (HD + 1)].rearrange(
                        "p (h c) -> p h c", c=HD + 1)
                    for j in range(4):
                        h = hh * 4 + j
                        nc.tensor.matmul(
                            psc[:, j, :],
                            attn[:, h, :],
                            vt[:, h, :],
                            start=True, stop=True)
                    rr = p_small.tile([128, 4], F32, tag="rr")
                    nc.vector.reciprocal(rr[:], psc[:, :, HD])
                    if hh == 0:
                        ctx = p_ctx.tile([128, H, HD], BF16, tag="ctx")
                    nc.vector.tensor_tensor(
                        ctx[:, hh * 4:hh * 4 + 4, :],
                        psc[:, :, 0:HD],
                        rr[:, :, None].to_broadcast((128, 4, HD)),
                        mybir.AluOpType.mult)

                if stage <= 5:
                    continue
                # ctx^T via PE transposes
                pst = ps_b1.tile([128, D], BF16, tag="b1")
                cflat = ctx[:].rearrange("p h c -> p (h c)")
                for ec in range(DC):
                    nc.tensor.transpose(
                        pst[:, ec * 128:(ec + 1) * 128],
                        cflat[:, ec * 128:(ec + 1) * 128],
                        ident_bf[:])
                ctxt = p_ctxt.tile([128, DC, 128], BF16, tag="ctxt")
                if b % 2 == 0:
                    nc.vector.tensor_copy(
                        ctxt[:].rearrange("p d n -> p (d n)"), pst[:])
                else:
                    nc.scalar.copy(
                        ctxt[:].rearrange("p d n -> p (d n)"), pst[:])

                if stage <= 6:
                    continue
                # out projection (natural [n, e]) + gate column
                pso = ps_b1.tile([128, D], F32, tag="b1")
                for dc in range(DC):
                    nc.tensor.matmul(
                        pso[:], ctxt[:, dc, :], wo_sb[:, dc, :],
                        start=(dc == 0), stop=(dc == DC - 1))
                for dc in range(DC):
                    nc.tensor.matmul(
                        psg[:, 32 + b:33 + b], ctxt[:, dc, :], weff_sb[:, dc, :],
                        start=(dc == 0), stop=(dc == DC - 1))
                ao = p_ao.tile([128, D], BF16, tag="ao")
                if has_bo:
                    nc.vector.tensor_tensor(
                        ao[:], pso[:], bo_full[:], mybir.AluOpType.add)
                else:
                    nc.scalar.copy(ao[:], pso[:])
                ao_sbs.append(ao)

            if stage <= 7:
                continue
            # gates for the whole superblock
            tsb = p_small.tile([128, 4], F32, tag="tsb")
            tanhf = (mybir.ActivationFunctionType.Copy
                     if "noexp" in variant else
                     mybir.ActivationFunctionType.Tanh)
            if "noexp" in variant:
                nc.scalar.activation(tsb[:], psg[:, 32:36], tanhf, scale=0.5)
            else:
                nc.scalar.activation(
                    tsb[:], psg[:, 32:36], tanhf,
                    bias=(gbh_full[:] if has_gb else 0.0), scale=0.5)
            gsb = p_small.tile([128, 4], BF16, tag="gsb")
            nc.vector.tensor_scalar(
                gsb[:], tsb[:], 0.5, 0.5,
                mybir.AluOpType.mult, mybir.AluOpType.add)
            G = p_small.tile([128, 8], BF16, tag="G")
            nc.vector.memset(G[:], 0.0)
            Gv = G[:].rearrange("p (a t) -> p a t", t=2)
            nc.vector.tensor_copy(Gv[0:64, :, 0], gsb[0:64, :])
            nc.vector.tensor_copy(Gv[64:128, :, 1], gsb[64:128, :])

            # readout: out^T[e, g] accumulated per (block, e-chunk)
            rv = psg[:, 0:32].rearrange("p (e g) -> p e g", e=DC)
            for b in range(NBLK):
                ao = ao_sbs[b]
                for ec in range(DC):
                    nc.tensor.matmul(
                        rv[:, ec, 2 * b:2 * b + 2],
                        ao[:, ec * 128:(ec + 1) * 128],
                        G[:, 2 * b:2 * b + 2],
                        start=True, stop=True)
            nc.vector.tensor_copy(outT[:, :, sb * 8:(sb + 1) * 8], rv)

        # ---- phase C: final transpose of outT -> out [g, e] ----
        if stage >= 8:
            psf = ps_b1.tile([128, D], F32, tag="b1", name="psf")
        if stage >= 8:
            for dc in range(DC):
                nc.tensor.transpose(
                    psf[0:gc, dc * 128:(dc + 1) * 128], outT[:, dc, :],
                    ident_f32[:])
            out_sb = p_small.tile([128, D], F32, tag="osb")
            nc.vector.tensor_copy(out_sb[0:gc, :], psf[0:gc, :])
            nc.sync.dma_start(out[:, :], out_sb[0:gc, :])

    import time as _time
    _t = _time.time()
    nc.compile()
    print(f"[kernel] bacc compile: {_time.time()-_t:.1f}s", flush=True)
    return nc


def kernel(x, batch, in_proj_w, in_proj_b, out_proj_w, out_proj_b,
           gate_w, gate_b):
    x = np.ascontiguousarray(np.asarray(x, dtype=np.float32))
    in_proj_w = np.asarray(in_proj_w, dtype=np.float32)
    in_proj_b = np.asarray(in_proj_b, dtype=np.float32)
    out_proj_w = np.asarray(out_proj_w, dtype=np.float32)
    out_proj_b = np.asarray(out_proj_b, dtype=np.float32)
    gate_w = np.asarray(gate_w, dtype=np.float32)
    gate_b = np.asarray(gate_b, dtype=np.float32)

    # host-side weight prep
    wqkT = in_proj_w[:2 * D].T                              # [512, 1024]
    wqk_h = np.ascontiguousarray(
        wqkT.reshape(DC, 128, 2 * D).transpose(1, 0, 2)).astype(ml_dtypes.bfloat16)
    wvT = in_proj_w[2 * D:].T                               # [512, 512]
    wv_h = np.ascontiguousarray(
        wvT.reshape(DC, 128, D).transpose(1, 0, 2)).astype(ml_dtypes.bfloat16)
    woT = out_proj_w.T                                      # [512, 512]
    wo_h = np.ascontiguousarray(
        woT.reshape(DC, 128, D).transpose(1, 0, 2)).astype(ml_dtypes.bfloat16)
    weff = (out_proj_w.T @ gate_w[0]).astype(np.float32)    # [512]
    weff_h = np.ascontiguousarray(
        weff.reshape(DC, 128, 1).transpose(1, 0, 2)).astype(ml_dtypes.bfloat16)

    bqk_np = in_proj_b[:2 * D]
    bv_np = in_proj_b[2 * D:]
    gb_eff = float(gate_b[0] + out_proj_b @ gate_w[0])
    has_bqk = bool(np.any(bqk_np))
    has_bv = bool(np.any(bv_np))
    has_bo = bool(np.any(out_proj_b))
    has_gb = gb_eff != 0.0

    import time as _time
    _t = _time.time()
    nc = _build(has_bqk, has_bv, has_bo, has_gb)
    print(f"[kernel] build total: {_time.time()-_t:.1f}s", flush=True)

    in_maps = []
    for c in range(N_CORES):
        m = {
            "xbf": np.ascontiguousarray(
                x[c * ROWS:(c + 1) * ROWS]).astype(ml_dtypes.bfloat16),
            "wqk": wqk_h, "wv": wv_h, "wo": wo_h, "weff": weff_h,
        }
        if has_bqk:
            m["bqk"] = np.ascontiguousarray(
                bqk_np.reshape(2 * DC, 128).T).astype(np.float32)
        if has_bv:
            m["bv"] = bv_np.reshape(1, D).astype(np.float32)
        if has_bo:
            m["bo"] = out_proj_b.reshape(1, D).astype(np.float32)
        if has_gb:
            m["gbh"] = np.array([[0.5 * gb_eff]], dtype=np.float32)
        in_maps.append(m)

    kernel.last_nc = nc
    kernel.last_in_maps = in_maps
    kernel.last_flags = (has_bqk, has_bv, has_bo, has_gb)

    res = run_bass_kernel_spmd(
        nc, in_maps, core_ids=list(range(N_CORES)), trace=TRACE)
    if TRACE:
        kernel.last_exec_time_ns = res.exec_time_ns
        kernel.last_results = res

    return np.concatenate([r["out"] for r in res.results], axis=0)


kernel.last_exec_time_ns = None
kernel.last_results = None
kernel.last_nc = None
kernel.last_in_maps = None


def _make_runner(nc, in_maps):
    """Build a repeat-callable PJRT runner for `nc` with device-resident
    inputs (mirrors bass2jax.run_bass_via_pjrt's multi-core path, minus
    output donation so buffers can be reused across timing iterations)."""
    import jax
    from jax.sharding import Mesh, PartitionSpec, NamedSharding
    from jax.experimental.shard_map import shard_map
    from concourse import bass2jax

    bass2jax.install_neuronx_cc_hook()
    n_cores = len(in_maps)

    partition_name = (
        nc.partition_id_tensor.name if nc.partition_id_tensor else None)
    in_names, out_names, out_avals, zero_outs = [], [], [], []
    for alloc in nc.m.functions[0].allocations:
        if not isinstance(alloc, mybir.MemoryLocationSet):
            continue
        name = alloc.memorylocations[0].name
        if alloc.kind == "ExternalInput":
            if name != partition_name:
                in_names.append(name)
        elif alloc.kind == "ExternalOutput":
            shape = tuple(alloc.tensor_shape)
            dtype = mybir.dt.np(alloc.dtype)
            out_avals.append(jax.core.ShapedArray(shape, dtype))
            out_names.append(name)
            zero_outs.append(np.zeros(shape, dtype))
    n_params = len(in_names)
    all_in_names = in_names + out_names
    if partition_name is not None:
        all_in_names = all_in_names + [partition_name]

    def _body(*args):
        operands = list(args)
        if partition_name is not None:
            operands.append(bass2jax.partition_id_tensor())
        outs = bass2jax._bass_exec_p.bind(
            *operands,
            out_avals=tuple(out_avals),
            in_names=tuple(all_in_names),
            out_names=tuple(out_names),
            lowering_input_output_aliases=(),
            sim_require_finite=True,
            sim_require_nnan=True,
            nc=nc,
        )
        return tuple(outs)

    devices = jax.devices()[:n_cores]
    mesh = Mesh(np.asarray(devices), ("core",))
    nsp = len(in_names) + len(out_names)
    sharded = jax.jit(
        shard_map(_body, mesh=mesh,
                  in_specs=(PartitionSpec("core"),) * nsp,
                  out_specs=(PartitionSpec("core"),) * len(out_names),
                  check_rep=False),
        keep_unused=True,
    )
    sharding = NamedSharding(mesh, PartitionSpec("core"))
    concat_in = [
        np.concatenate([np.asarray(in_maps[c][name]) for c in range(n_cores)], axis=0)
        for name in in_names
    ] + [np.zeros((n_cores * z.shape[0], *z.shape[1:]), z.dtype) for z in zero_outs]
    dev_in = [jax.device_put(a, sharding) for a in concat_in]

    def run_once(block=True):
        outs = sharded(*dev_in)
        if block:
            jax.block_until_ready(outs)
        return outs

    return run_once


def _build_empty():
    """Tiny program for measuring per-dispatch overhead."""
    nc = bacc.Bacc(None, target_bir_lowering=False, debug=False)
    a = nc.dram_tensor("a", [128, 128], F32, kind="ExternalInput")
    o = nc.dram_tensor("o", [128, 128], F32, kind="ExternalOutput")
    with tile.TileContext(nc) as tc:
        with tc.tile_pool(name="sb", bufs=1) as sb:
            t = sb.tile([128, 128], F32, tag="t")
            nc.sync.dma_start(t[:], a[:])
            nc.sync.dma_start(o[:, :], t[:])
    nc.compile()
    return nc


def bench(iters=20, warmup=3):
    """Timing of the last-built kernel.

    The axon dispatch path quantizes blocking-call wall time (~75 ms), so a
    single execution cannot be resolved directly.  Instead build a variant
    of the same program with the whole per-superblock pipeline repeated
    twice ("x2") and report min(T_x2) - min(T_x1): the marginal cost of one
    full compute pass, with all dispatch overhead cancelled.
    """
    import time
    assert kernel.last_nc is not None, "call kernel() first"

    runner = _make_runner(kernel.last_nc, kernel.last_in_maps)
    nc_x2 = _build(*kernel.last_flags, variant=("x2",))
    runner_x2 = _make_runner(nc_x2, kernel.last_in_maps)

    def measure(run):
        for _ in range(warmup):
            run()
        ts = []
        for _ in range(iters):
            t0 = time.perf_counter()
            run()
            ts.append(time.perf_counter() - t0)
        ts.sort()
        return ts

    ts_k = measure(runner)
    ts_2 = measure(runner_x2)
    print(f"[bench] x1 min/p25/med: "
          f"{ts_k[0]*1e6:.0f}/{ts_k[len(ts_k)//4]*1e6:.0f}/"
          f"{ts_k[len(ts_k)//2]*1e6:.0f} us")
    print(f"[bench] x2 min/p25/med: "
          f"{ts_2[0]*1e6:.0f}/{ts_2[len(ts_2)//4]*1e6:.0f}/"
          f"{ts_2[len(ts_2)//2]*1e6:.0f} us")
    exec_ns = (ts_2[0] - ts_k[0]) * 1e9
    return exec_ns, ts_k[0] * 1e9, ts_2[0] * 1e9



# revision 7
# speedup vs baseline: 4.2383x; 1.1853x over previous
"""AttentionReadout Trainium2 kernel (8-core data-parallel over the graph axis).

Reference computation (per graph of 64 nodes, D=512, H=8 heads, hd=64):
    qkv = x @ in_proj_w.T + in_proj_b ; q,k,v = split(qkv)
    attn = softmax(q k^T / sqrt(hd)) v          (per head)
    attn_out = attn @ out_proj_w.T + out_proj_b
    gates = sigmoid(attn_out @ gate_w.T + gate_b)
    out[g] = sum_n attn_out[n] * gates[n]

v2 layout (per core: 128 graphs = 8192 nodes, superblocks of 512 nodes):
  - X^T tiles ([d,n], bf16) via DMA-xbar transpose.
  - Q^T,K^T projected in [e,n] orientation; odd heads' rows live at
    partitions 64:127 and are used IN PLACE via tile_position row band 64
    (no SBUF realign DMA).
  - V natural [n,e] + ones column (ctx matmul also yields softmax denom).
  - Scores S^T[m,n] for all 8 heads of a 128-node block in one 2-bank psum
    tile; exp on ScalarE in 2 quadrant ops covering all heads.
  - ctx + rowsum in one 2-bank psum tile; one reciprocal + one multiply.
  - ctx^T via PE transposes; out projection natural [n,e]; gate column via
    w_eff = out_proj_w.T@gw (reusing ctx^T stationary).
  - sigmoid as 0.5*tanh(x/2)+0.5 folded into per-half gate writes into
    per-block gate matrices G_b [128,8] (zeros persistent).
  - readout: 4 accumulating matmuls (G_b^T @ ao) -> [8,512] psum, evac,
    per-superblock DMA straight to the output rows.
"""

import numpy as np
import ml_dtypes

import concourse.bass as bass
import concourse.mybir as mybir
import concourse.tile as tile
from concourse import bacc
from concourse.bass_utils import run_bass_kernel_spmd
from concourse.masks import make_identity

F32 = mybir.dt.float32
BF16 = mybir.dt.bfloat16

N_CORES = 8
D = 512
H = 8
HD = 64
NPG = 64            # nodes per graph
TOTAL = 65536
ROWS = TOTAL // N_CORES      # 8192 nodes per core
GC = ROWS // NPG             # 128 graphs per core
BLK = 128                    # nodes per block (2 graphs)
SBN = 512                    # nodes per superblock (4 blocks, 8 graphs)
NSB = ROWS // SBN            # 16 superblocks
NBLK = SBN // BLK            # 4 blocks per superblock
DC = D // 128                # 4 d-chunks

# module-level switch used by test.py; harness default is no tracing
TRACE = False

try:
    import jax as _jax
    _jax.config.update("jax_compilation_cache_dir", "/tmp/jax_neff_cache")
    _jax.config.update("jax_persistent_cache_min_compile_time_secs", 10)
    _jax.config.update("jax_persistent_cache_min_entry_size_bytes", 0)
except Exception:
    pass


def _build(has_bqk, has_bv, has_bo, has_gb, rows=ROWS, variant=()):
    variant = set(variant)
    reps = 1
    stage = 8
    for _v in variant:
        if _v.startswith("x") and _v[1:].isdigit():
            reps = int(_v[1:])
        if _v.startswith("s") and _v[1:].isdigit():
            stage = int(_v[1:])
    nsb = rows // SBN
    gc = rows // NPG
    nc = bacc.Bacc(None, target_bir_lowering=False, debug=False)

    xbf = nc.dram_tensor("xbf", [rows, D], BF16, kind="ExternalInput")
    wqk = nc.dram_tensor("wqk", [128, DC, 2 * D], BF16, kind="ExternalInput")
    wv = nc.dram_tensor("wv", [128, DC, D], BF16, kind="ExternalInput")
    wo = nc.dram_tensor("wo", [128, DC, D], BF16, kind="ExternalInput")
    weff = nc.dram_tensor("weff", [128, DC, 1], BF16, kind="ExternalInput")
    if has_bqk:
        bqk = nc.dram_tensor("bqk", [128, 2 * DC], F32, kind="ExternalInput")
    if has_bv:
        bv = nc.dram_tensor("bv", [1, D], F32, kind="ExternalInput")
    if has_bo:
        bo = nc.dram_tensor("bo", [1, D], F32, kind="ExternalInput")
    if has_gb:
        gbh = nc.dram_tensor("gbh", [1, 1], F32, kind="ExternalInput")
    out = nc.dram_tensor("out", [gc, D], F32, kind="ExternalOutput")

    from contextlib import ExitStack
    with tile.TileContext(nc) as tc, ExitStack() as st:
        consts = st.enter_context(tc.tile_pool(name="consts", bufs=1))
        p_xt = st.enter_context(tc.tile_pool(name="p_xt", bufs=3))
        p_qkt = st.enter_context(tc.tile_pool(name="p_qkt", bufs=2))
        p_v = st.enter_context(tc.tile_pool(name="p_v", bufs=4))
        p_attn = st.enter_context(tc.tile_pool(name="p_attn", bufs=3))
        p_ctx = st.enter_context(tc.tile_pool(name="p_ctx", bufs=3))
        p_ctxt = st.enter_context(tc.tile_pool(name="p_ctxt", bufs=3))
        p_ao = st.enter_context(tc.tile_pool(name="p_ao", bufs=6))
        p_small = st.enter_context(tc.tile_pool(name="p_small", bufs=4))
        # PSUM budget (8 banks): a=2, s=2, c=2, b=2
        ps_a = st.enter_context(tc.tile_pool(name="ps_a", bufs=2, space="PSUM"))
        ps_s = st.enter_context(tc.tile_pool(name="ps_s", bufs=1, space="PSUM"))
        ps_c = st.enter_context(tc.tile_pool(name="ps_c", bufs=1, space="PSUM"))
        ps_b = st.enter_context(tc.tile_pool(name="ps_b", bufs=2, space="PSUM"))

        # ---- constants / weights ----
        ident_bf = consts.tile([128, 128], BF16, tag="ident_bf")
        make_identity(nc, ident_bf[:])

        wqk_sb = consts.tile([128, DC, 2 * D], BF16, tag="wqk")
        nc.sync.dma_start(wqk_sb[:], wqk[:, :, :])
        wv_sb = consts.tile([128, DC, D], BF16, tag="wv")
        nc.sync.dma_start(wv_sb[:], wv[:, :, :])
        wo_sb = consts.tile([128, DC, D], BF16, tag="wo")
        nc.sync.dma_start(wo_sb[:], wo[:, :, :])
        weff_sb = consts.tile([128, DC, 1], BF16, tag="weff")
        nc.sync.dma_start(weff_sb[:], weff[:, :, :])

        if has_bqk:
            bqk_sb = consts.tile([128, 2 * DC], F32, tag="bqk")
            nc.sync.dma_start(bqk_sb[:], bqk[:, :])
        if has_bv:
            bv_row = consts.tile([1, D], F32, tag="bv_row")
            nc.sync.dma_start(bv_row[:], bv[:, :])
            bv_full = consts.tile([128, D], F32, tag="bv_full")
            nc.gpsimd.partition_broadcast(bv_full[:], bv_row[:])
        if has_bo:
            bo_row = consts.tile([1, D], F32, tag="bo_row")
            nc.sync.dma_start(bo_row[:], bo[:, :])
            bo_full = consts.tile([128, D], F32, tag="bo_full")
            nc.gpsimd.partition_broadcast(bo_full[:], bo_row[:])
        if has_gb:
            gbh_row = consts.tile([1, 1], F32, tag="gbh_row")
            nc.sync.dma_start(gbh_row[:], gbh[:, :])
            gbh_full = consts.tile([128, 1], F32, tag="gbh_full")
            nc.gpsimd.partition_broadcast(gbh_full[:], gbh_row[:])

        # per-block gate matrices: only cols 2b,2b+1 ever written (half
        # columns each); the zero elsewhere persists across superblocks.
        Gb = []
        for b in range(NBLK):
            g = consts.tile([128, 2 * NBLK], BF16, tag=f"G{b}")
            nc.vector.memset(g[:], 0.0)
            Gb.append(g)

        # ---- main loop: per superblock ----
        for sb in list(range(nsb)) * reps:
            r0 = sb * SBN

            # X^T tiles [d_part, dc, n]
            xt = p_xt.tile([128, DC, SBN], BF16, tag="xt")
            for dc in range(DC):
                nc.sync.dma_start_transpose(
                    xt[:, dc, :], xbf[r0:r0 + SBN, dc * 128:(dc + 1) * 128])

            # Q^T,K^T projection: [e, n] orientation, 8 e-chunks of 128
            qkt = p_qkt.tile([128, 8, SBN], BF16, tag="qkt")
            for ec in range(8):
                ps = ps_a.tile([128, SBN], F32, tag="a")
                for dc in range(DC):
                    nc.tensor.matmul(
                        ps[:],
                        wqk_sb[:, dc, ec * 128:(ec + 1) * 128],
                        xt[:, dc, :],
                        start=(dc == 0), stop=(dc == DC - 1))
                if has_bqk:
                    if ec % 2 == 0:
                        nc.vector.tensor_scalar_add(
                            qkt[:, ec, :], ps[:], bqk_sb[:, ec:ec + 1])
                    else:
                        nc.scalar.activation(
                            qkt[:, ec, :], ps[:],
                            mybir.ActivationFunctionType.Identity,
                            bias=bqk_sb[:, ec:ec + 1])
                else:
                    if ec % 2 == 0:
                        nc.vector.tensor_copy(qkt[:, ec, :], ps[:])
                    else:
                        nc.scalar.copy(qkt[:, ec, :], ps[:])

            if stage <= 1:
                continue
            # V projection (natural [n, e]) per block, with ones column
            v_sbs = []
            for b in range(NBLK):
                ps = ps_a.tile([128, SBN], F32, tag="a")
                for dc in range(DC):
                    nc.tensor.matmul(
                        ps[:, 0:D],
                        xt[:, dc, b * 128:(b + 1) * 128],
                        wv_sb[:, dc, :],
                        start=(dc == 0), stop=(dc == DC - 1))
                vt = p_v.tile([128, H, HD + 1], BF16, tag="v")
                pv = ps[:, 0:D].rearrange("p (h c) -> p h c", h=H)
                if has_bv:
                    nc.vector.tensor_tensor(
                        vt[:, :, 0:HD], pv,
                        bv_full[:].rearrange("p (h c) -> p h c", h=H),
                        mybir.AluOpType.add)
                else:
                    if b % 2 == 0:
                        nc.vector.tensor_copy(vt[:, :, 0:HD], pv)
                    else:
                        nc.scalar.copy(vt[:, :, 0:HD], pv)
                nc.vector.memset(vt[:, :, HD:HD + 1], 1.0)
                v_sbs.append(vt)

            if stage <= 2:
                continue
            # per block: attention + out projection + gates
            ao_sbs = []
            for b in range(NBLK):
                n0 = b * 128
                vt = v_sbs[b]

                # scores S^T[m, n], all 8 heads, one 2-bank psum tile
                # head h = 2j + band; band-64 heads write psum bank 1 so
                # concurrently-running sub-array matmuls never share a bank
                pss_t = ps_s.tile([128, 2, SBN], F32, tag="s")
                pss = pss_t[:].rearrange("p g (j n) -> p g j n", j=4)
                for h in range(H):
                    p0 = (h % 2) * 64
                    nc.tensor.matmul(
                        pss[:, h % 2, h // 2, :],
                        qkt[p0:p0 + 64, 4 + h // 2, n0:n0 + 128],
                        qkt[p0:p0 + 64, h // 2, n0:n0 + 128],
                        start=True, stop=True)

                # exp of the two valid 64x64 quadrant sets (scale 1/sqrt(hd))
                attn = p_attn.tile([128, H, BLK], BF16, tag="attn")
                nc.gpsimd.memset(attn[0:64, :, 64:128], 0.0)
                nc.gpsimd.memset(attn[64:128, :, 0:64], 0.0)
                av = attn[:].rearrange("p (j g) n -> p g j n", g=2)
                nc.scalar.activation(
                    av[0:64, :, :, 0:64], pss[0:64, :, :, 0:64],
                    mybir.ActivationFunctionType.Exp, scale=0.125)
                nc.scalar.activation(
                    av[64:128, :, :, 64:128], pss[64:128, :, :, 64:128],
                    mybir.ActivationFunctionType.Exp, scale=0.125)

                if stage <= 3:
                    continue
                # ctx (+rowsum): [n, hd+1] per head, one 2-bank psum tile
                psc_t = ps_c.tile([128, 2, SBN], F32, tag="c")
                psc = psc_t[:, :, 0:4 * (HD + 1)].rearrange(
                    "p g (j c) -> p g j c", c=HD + 1)
                for h in range(H):
                    nc.tensor.matmul(
                        psc[:, h // 4, h % 4, :],
                        attn[:, h, :],
                        vt[:, h, :],
                        start=True, stop=True)
                rr = p_small.tile([128, 2, 4], F32, tag="rr")
                nc.vector.reciprocal(rr[:], psc[:, :, :, HD])
                ctx = p_ctx.tile([128, H, HD], BF16, tag="ctx")
                cv = ctx[:].rearrange("p (g j) c -> p g j c", g=2)
                nc.vector.tensor_tensor(
                    cv[:], psc[:, :, :, 0:HD],
                    rr[:, :, :, None].to_broadcast((128, 2, 4, HD)),
                    mybir.AluOpType.mult)

                if stage <= 4:
                    continue
                # ctx^T via PE transposes
                pst = ps_b.tile([128, D], BF16, tag="b")
                cflat = ctx[:].rearrange("p h c -> p (h c)")
                for ec in range(DC):
                    nc.tensor.transpose(
                        pst[:, ec * 128:(ec + 1) * 128],
                        cflat[:, ec * 128:(ec + 1) * 128],
                        ident_bf[:])
                ctxt = p_ctxt.tile([128, DC, 128], BF16, tag="ctxt")
                if b % 2 == 0:
                    nc.vector.tensor_copy(
                        ctxt[:].rearrange("p d n -> p (d n)"), pst[:])
                else:
                    nc.scalar.copy(
                        ctxt[:].rearrange("p d n -> p (d n)"), pst[:])

                if stage <= 5:
                    continue
                # out projection (natural [n, e]) + gate-logit column
                pso = ps_b.tile([128, D], F32, tag="b")
                for dc in range(DC):
                    nc.tensor.matmul(
                        pso[:], ctxt[:, dc, :], wo_sb[:, dc, :],
                        start=(dc == 0), stop=(dc == DC - 1))
                psg = ps_b.tile([128, 1], F32, tag="b")
                for dc in range(DC):
                    nc.tensor.matmul(
                        psg[:], ctxt[:, dc, :], weff_sb[:, dc, :],
                        start=(dc == 0), stop=(dc == DC - 1))
                ao = p_ao.tile([128, D], BF16, tag="ao")
                if has_bo:
                    nc.vector.tensor_tensor(
                        ao[:], pso[:], bo_full[:], mybir.AluOpType.add)
                else:
                    if b % 2 == 0:
                        nc.scalar.copy(ao[:], pso[:])
                    else:
                        nc.vector.tensor_copy(ao[:], pso[:])
                ao_sbs.append(ao)

                if stage <= 6:
                    continue
                # gates: sigmoid(x) = 0.5*tanh(x/2) + 0.5, written into the
                # two live half-columns of G_b
                tb = p_small.tile([128, 1], F32, tag="tb")
                nc.scalar.activation(
                    tb[:], psg[:],
                    mybir.ActivationFunctionType.Tanh,
                    bias=(gbh_full[:] if has_gb else 0.0), scale=0.5)
                nc.vector.tensor_scalar(
                    Gb[b][0:64, 2 * b:2 * b + 1], tb[0:64, :], 0.5, 0.5,
                    mybir.AluOpType.mult, mybir.AluOpType.add)
                nc.vector.tensor_scalar(
                    Gb[b][64:128, 2 * b + 1:2 * b + 2], tb[64:128, :],
                    0.5, 0.5,
                    mybir.AluOpType.mult, mybir.AluOpType.add)

            if stage <= 7:
                continue
            # readout: out[8 graphs, e] = sum_b G_b^T @ ao_b
            psro = ps_a.tile([8, SBN], F32, tag="a")
            for b in range(NBLK):
                nc.tensor.matmul(
                    psro[:], Gb[b][:, :], ao_sbs[b][:],
                    start=(b == 0), stop=(b == NBLK - 1))
            ro = p_small.tile([8, D], F32, tag="ro")
            if sb % 2 == 0:
                nc.vector.tensor_copy(ro[:], psro[:])
            else:
                nc.scalar.copy(ro[:], psro[:])
            nc.sync.dma_start(out[sb * 8:(sb + 1) * 8, :], ro[:])

    import time as _time
    _t = _time.time()
    nc.compile()
    print(f"[kernel] bacc compile: {_time.time()-_t:.1f}s", flush=True)
    return nc


def kernel(x, batch, in_proj_w, in_proj_b, out_proj_w, out_proj_b,
           gate_w, gate_b):
    x = np.ascontiguousarray(np.asarray(x, dtype=np.float32))
    in_proj_w = np.asarray(in_proj_w, dtype=np.float32)
    in_proj_b = np.asarray(in_proj_b, dtype=np.float32)
    out_proj_w = np.asarray(out_proj_w, dtype=np.float32)
    out_proj_b = np.asarray(out_proj_b, dtype=np.float32)
    gate_w = np.asarray(gate_w, dtype=np.float32)
    gate_b = np.asarray(gate_b, dtype=np.float32)

    # host-side weight prep
    wqkT = in_proj_w[:2 * D].T                              # [512, 1024]
    wqk_h = np.ascontiguousarray(
        wqkT.reshape(DC, 128, 2 * D).transpose(1, 0, 2)).astype(ml_dtypes.bfloat16)
    wvT = in_proj_w[2 * D:].T                               # [512, 512]
    wv_h = np.ascontiguousarray(
        wvT.reshape(DC, 128, D).transpose(1, 0, 2)).astype(ml_dtypes.bfloat16)
    woT = out_proj_w.T                                      # [512, 512]
    wo_h = np.ascontiguousarray(
        woT.reshape(DC, 128, D).transpose(1, 0, 2)).astype(ml_dtypes.bfloat16)
    weff = (out_proj_w.T @ gate_w[0]).astype(np.float32)    # [512]
    weff_h = np.ascontiguousarray(
        weff.reshape(DC, 128, 1).transpose(1, 0, 2)).astype(ml_dtypes.bfloat16)

    bqk_np = in_proj_b[:2 * D]
    bv_np = in_proj_b[2 * D:]
    gb_eff = float(gate_b[0] + out_proj_b @ gate_w[0])
    has_bqk = bool(np.any(bqk_np))
    has_bv = bool(np.any(bv_np))
    has_bo = bool(np.any(out_proj_b))
    has_gb = gb_eff != 0.0

    import time as _time
    _t = _time.time()
    nc = _build(has_bqk, has_bv, has_bo, has_gb)
    print(f"[kernel] build total: {_time.time()-_t:.1f}s", flush=True)

    in_maps = []
    for c in range(N_CORES):
        m = {
            "xbf": np.ascontiguousarray(
                x[c * ROWS:(c + 1) * ROWS]).astype(ml_dtypes.bfloat16),
            "wqk": wqk_h, "wv": wv_h, "wo": wo_h, "weff": weff_h,
        }
        if has_bqk:
            m["bqk"] = np.ascontiguousarray(
                bqk_np.reshape(2 * DC, 128).T).astype(np.float32)
        if has_bv:
            m["bv"] = bv_np.reshape(1, D).astype(np.float32)
        if has_bo:
            m["bo"] = out_proj_b.reshape(1, D).astype(np.float32)
        if has_gb:
            m["gbh"] = np.array([[0.5 * gb_eff]], dtype=np.float32)
        in_maps.append(m)

    kernel.last_nc = nc
    kernel.last_in_maps = in_maps
    kernel.last_flags = (has_bqk, has_bv, has_bo, has_gb)

    res = run_bass_kernel_spmd(
        nc, in_maps, core_ids=list(range(N_CORES)), trace=TRACE)
    if TRACE:
        kernel.last_exec_time_ns = res.exec_time_ns
        kernel.last_results = res

    return np.concatenate([r["out"] for r in res.results], axis=0)


kernel.last_exec_time_ns = None
kernel.last_results = None
kernel.last_nc = None
kernel.last_in_maps = None


def _make_runner(nc, in_maps):
    """Build a repeat-callable PJRT runner for `nc` with device-resident
    inputs (mirrors bass2jax.run_bass_via_pjrt's multi-core path, minus
    output donation so buffers can be reused across timing iterations)."""
    import jax
    from jax.sharding import Mesh, PartitionSpec, NamedSharding
    from jax.experimental.shard_map import shard_map
    from concourse import bass2jax

    bass2jax.install_neuronx_cc_hook()
    n_cores = len(in_maps)

    partition_name = (
        nc.partition_id_tensor.name if nc.partition_id_tensor else None)
    in_names, out_names, out_avals, zero_outs = [], [], [], []
    for alloc in nc.m.functions[0].allocations:
        if not isinstance(alloc, mybir.MemoryLocationSet):
            continue
        name = alloc.memorylocations[0].name
        if alloc.kind == "ExternalInput":
            if name != partition_name:
                in_names.append(name)
        elif alloc.kind == "ExternalOutput":
            shape = tuple(alloc.tensor_shape)
            dtype = mybir.dt.np(alloc.dtype)
            out_avals.append(jax.core.ShapedArray(shape, dtype))
            out_names.append(name)
            zero_outs.append(np.zeros(shape, dtype))
    all_in_names = in_names + out_names
    if partition_name is not None:
        all_in_names = all_in_names + [partition_name]

    def _body(*args):
        operands = list(args)
        if partition_name is not None:
            operands.append(bass2jax.partition_id_tensor())
        outs = bass2jax._bass_exec_p.bind(
            *operands,
            out_avals=tuple(out_avals),
            in_names=tuple(all_in_names),
            out_names=tuple(out_names),
            lowering_input_output_aliases=(),
            sim_require_finite=True,
            sim_require_nnan=True,
            nc=nc,
        )
        return tuple(outs)

    devices = jax.devices()[:n_cores]
    mesh = Mesh(np.asarray(devices), ("core",))
    nsp = len(in_names) + len(out_names)
    sharded = jax.jit(
        shard_map(_body, mesh=mesh,
                  in_specs=(PartitionSpec("core"),) * nsp,
                  out_specs=(PartitionSpec("core"),) * len(out_names),
                  check_rep=False),
        keep_unused=True,
    )
    sharding = NamedSharding(mesh, PartitionSpec("core"))
    concat_in = [
        np.concatenate([np.asarray(in_maps[c][name]) for c in range(n_cores)], axis=0)
        for name in in_names
    ] + [np.zeros((n_cores * z.shape[0], *z.shape[1:]), z.dtype) for z in zero_outs]
    dev_in = [jax.device_put(a, sharding) for a in concat_in]

    def run_once(block=True):
        outs = sharded(*dev_in)
        if block:
            jax.block_until_ready(outs)
        return outs

    return run_once


def bench(iters=20, warmup=3):
    """Wall-clock timing is dispatch-dominated (~73 ms) and cannot resolve
    device time; kept for compatibility.  Reports x2-x1 marginal."""
    import time
    assert kernel.last_nc is not None, "call kernel() first"

    runner = _make_runner(kernel.last_nc, kernel.last_in_maps)
    nc_x2 = _build(*kernel.last_flags, variant=("x2",))
    runner_x2 = _make_runner(nc_x2, kernel.last_in_maps)

    def measure(run):
        for _ in range(warmup):
            run()
        ts = []
        for _ in range(iters):
            t0 = time.perf_counter()
            run()
            ts.append(time.perf_counter() - t0)
        ts.sort()
        return ts

    ts_k = measure(runner)
    ts_2 = measure(runner_x2)
    exec_ns = (ts_2[0] - ts_k[0]) * 1e9
    return exec_ns, ts_k[0] * 1e9, ts_2[0] * 1e9


# revision 8
# speedup vs baseline: 4.6774x; 1.1036x over previous
"""AttentionReadout Trainium2 kernel (8-core data-parallel over the graph axis).

Reference computation (per graph of 64 nodes, D=512, H=8 heads, hd=64):
    qkv = x @ in_proj_w.T + in_proj_b ; q,k,v = split(qkv)
    attn = softmax(q k^T / sqrt(hd)) v          (per head)
    attn_out = attn @ out_proj_w.T + out_proj_b
    gates = sigmoid(attn_out @ gate_w.T + gate_b)
    out[g] = sum_n attn_out[n] * gates[n]

v2 layout (per core: 128 graphs = 8192 nodes, superblocks of 512 nodes):
  - X^T tiles ([d,n], bf16) via DMA-xbar transpose.
  - Q^T,K^T projected in [e,n] orientation; odd heads' rows live at
    partitions 64:127 and are used IN PLACE via tile_position row band 64
    (no SBUF realign DMA).
  - V natural [n,e] + ones column (ctx matmul also yields softmax denom).
  - Scores S^T[m,n] for all 8 heads of a 128-node block in one 2-bank psum
    tile; exp on ScalarE in 2 quadrant ops covering all heads.
  - ctx + rowsum in one 2-bank psum tile; one reciprocal + one multiply.
  - ctx^T via PE transposes; out projection natural [n,e]; gate column via
    w_eff = out_proj_w.T@gw (reusing ctx^T stationary).
  - sigmoid as 0.5*tanh(x/2)+0.5 folded into per-half gate writes into
    per-block gate matrices G_b [128,8] (zeros persistent).
  - readout: 4 accumulating matmuls (G_b^T @ ao) -> [8,512] psum, evac,
    per-superblock DMA straight to the output rows.
"""

import numpy as np
import ml_dtypes

import concourse.bass as bass
import concourse.mybir as mybir
import concourse.tile as tile
from concourse import bacc
from concourse.bass_utils import run_bass_kernel_spmd
from concourse.masks import make_identity

F32 = mybir.dt.float32
BF16 = mybir.dt.bfloat16

N_CORES = 8
D = 512
H = 8
HD = 64
NPG = 64            # nodes per graph
TOTAL = 65536
ROWS = TOTAL // N_CORES      # 8192 nodes per core
GC = ROWS // NPG             # 128 graphs per core
BLK = 128                    # nodes per block (2 graphs)
SBN = 512                    # nodes per superblock (4 blocks, 8 graphs)
NSB = ROWS // SBN            # 16 superblocks
NBLK = SBN // BLK            # 4 blocks per superblock
DC = D // 128                # 4 d-chunks

# module-level switch used by test.py; harness default is no tracing
TRACE = False

try:
    import jax as _jax
    _jax.config.update("jax_compilation_cache_dir", "/tmp/jax_neff_cache")
    _jax.config.update("jax_persistent_cache_min_compile_time_secs", 10)
    _jax.config.update("jax_persistent_cache_min_entry_size_bytes", 0)
except Exception:
    pass


def _build(has_bqk, has_bv, has_bo, has_gb, rows=ROWS, variant=()):
    variant = set(variant)
    reps = 1
    stage = 8
    for _v in variant:
        if _v.startswith("x") and _v[1:].isdigit():
            reps = int(_v[1:])
        if _v.startswith("s") and _v[1:].isdigit():
            stage = int(_v[1:])
    nsb = rows // SBN
    gc = rows // NPG
    nc = bacc.Bacc(None, target_bir_lowering=False, debug=False)

    xbf = nc.dram_tensor("xbf", [rows, D], BF16, kind="ExternalInput")
    wqk = nc.dram_tensor("wqk", [128, DC, 2 * D], BF16, kind="ExternalInput")
    wv = nc.dram_tensor("wv", [128, DC, D], BF16, kind="ExternalInput")
    wo = nc.dram_tensor("wo", [128, DC, D], BF16, kind="ExternalInput")
    weff = nc.dram_tensor("weff", [128, DC, 1], BF16, kind="ExternalInput")
    if has_bqk:
        bqk = nc.dram_tensor("bqk", [128, 2 * DC], F32, kind="ExternalInput")
    if has_bv:
        bv = nc.dram_tensor("bv", [1, D], F32, kind="ExternalInput")
    if has_bo:
        bo = nc.dram_tensor("bo", [1, D], F32, kind="ExternalInput")
    if has_gb:
        gbh = nc.dram_tensor("gbh", [1, 1], F32, kind="ExternalInput")
    out = nc.dram_tensor("out", [gc, D], F32, kind="ExternalOutput")

    from contextlib import ExitStack
    with tile.TileContext(nc) as tc, ExitStack() as st:
        consts = st.enter_context(tc.tile_pool(name="consts", bufs=1))
        p_xt = st.enter_context(tc.tile_pool(name="p_xt", bufs=4))
        p_qkt = st.enter_context(tc.tile_pool(name="p_qkt", bufs=2))
        p_v = st.enter_context(tc.tile_pool(name="p_v", bufs=4))
        p_attn = st.enter_context(tc.tile_pool(name="p_attn", bufs=3))
        p_ctx = st.enter_context(tc.tile_pool(name="p_ctx", bufs=3))
        p_ctxt = st.enter_context(tc.tile_pool(name="p_ctxt", bufs=3))
        p_ao = st.enter_context(tc.tile_pool(name="p_ao", bufs=6))
        p_small = st.enter_context(tc.tile_pool(name="p_small", bufs=4))
        # PSUM budget (8 banks): a=2, s=2, c=2, b=2
        ps_a = st.enter_context(tc.tile_pool(name="ps_a", bufs=2, space="PSUM"))
        ps_s = st.enter_context(tc.tile_pool(name="ps_s", bufs=1, space="PSUM"))
        ps_c = st.enter_context(tc.tile_pool(name="ps_c", bufs=1, space="PSUM"))
        ps_b = st.enter_context(tc.tile_pool(name="ps_b", bufs=2, space="PSUM"))

        # ---- constants / weights ----
        ident_bf = consts.tile([128, 128], BF16, tag="ident_bf")
        make_identity(nc, ident_bf[:])

        wqk_sb = consts.tile([128, DC, 2 * D], BF16, tag="wqk")
        nc.scalar.dma_start(wqk_sb[:], wqk[:, :, :])
        wv_sb = consts.tile([128, DC, D], BF16, tag="wv")
        nc.scalar.dma_start(wv_sb[:], wv[:, :, :])
        wo_sb = consts.tile([128, DC, D], BF16, tag="wo")
        nc.scalar.dma_start(wo_sb[:], wo[:, :, :])
        weff_sb = consts.tile([128, DC, 1], BF16, tag="weff")
        nc.scalar.dma_start(weff_sb[:], weff[:, :, :])

        if has_bqk:
            bqk_sb = consts.tile([128, 2 * DC], F32, tag="bqk")
            nc.sync.dma_start(bqk_sb[:], bqk[:, :])
        if has_bv:
            bv_row = consts.tile([1, D], F32, tag="bv_row")
            nc.sync.dma_start(bv_row[:], bv[:, :])
            bv_full = consts.tile([128, D], F32, tag="bv_full")
            nc.gpsimd.partition_broadcast(bv_full[:], bv_row[:])
        if has_bo:
            bo_row = consts.tile([1, D], F32, tag="bo_row")
            nc.sync.dma_start(bo_row[:], bo[:, :])
            bo_full = consts.tile([128, D], F32, tag="bo_full")
            nc.gpsimd.partition_broadcast(bo_full[:], bo_row[:])
        if has_gb:
            gbh_row = consts.tile([1, 1], F32, tag="gbh_row")
            nc.sync.dma_start(gbh_row[:], gbh[:, :])
            gbh_full = consts.tile([128, 1], F32, tag="gbh_full")
            nc.gpsimd.partition_broadcast(gbh_full[:], gbh_row[:])

        # per-block gate matrices: only cols 2b,2b+1 ever written (half
        # columns each); the zero elsewhere persists across superblocks.
        Gb = []
        for b in range(NBLK):
            g = consts.tile([128, 2 * NBLK], BF16, tag=f"G{b}")
            nc.vector.memset(g[:], 0.0)
            Gb.append(g)

        # ---- main loop: per superblock ----
        for sb in list(range(nsb)) * reps:
            r0 = sb * SBN

            # X^T tiles [d_part, dc, n]
            xt = p_xt.tile([128, DC, SBN], BF16, tag="xt")
            for dc in range(DC):
                nc.sync.dma_start_transpose(
                    xt[:, dc, :], xbf[r0:r0 + SBN, dc * 128:(dc + 1) * 128])

            # Q^T,K^T projection: [e, n] orientation, 8 e-chunks of 128
            qkt = p_qkt.tile([128, 8, SBN], BF16, tag="qkt")
            for ec in range(8):
                ps = ps_a.tile([128, SBN], F32, tag="a")
                for dc in range(DC):
                    nc.tensor.matmul(
                        ps[:],
                        wqk_sb[:, dc, ec * 128:(ec + 1) * 128],
                        xt[:, dc, :],
                        start=(dc == 0), stop=(dc == DC - 1))
                if has_bqk:
                    if ec % 2 == 0:
                        nc.vector.tensor_scalar_add(
                            qkt[:, ec, :], ps[:], bqk_sb[:, ec:ec + 1])
                    else:
                        nc.scalar.activation(
                            qkt[:, ec, :], ps[:],
                            mybir.ActivationFunctionType.Identity,
                            bias=bqk_sb[:, ec:ec + 1])
                else:
                    if ec % 2 == 0:
                        nc.vector.tensor_copy(qkt[:, ec, :], ps[:])
                    else:
                        nc.scalar.copy(qkt[:, ec, :], ps[:])

            if stage <= 1:
                continue
            # V projection (natural [n, e]) per block, with ones column
            v_sbs = []
            for b in range(NBLK):
                ps = ps_a.tile([128, SBN], F32, tag="a")
                for dc in range(DC):
                    nc.tensor.matmul(
                        ps[:, 0:D],
                        xt[:, dc, b * 128:(b + 1) * 128],
                        wv_sb[:, dc, :],
                        start=(dc == 0), stop=(dc == DC - 1))
                vt = p_v.tile([128, H, HD + 1], BF16, tag="v")
                pv = ps[:, 0:D].rearrange("p (h c) -> p h c", h=H)
                if has_bv:
                    nc.vector.tensor_tensor(
                        vt[:, :, 0:HD], pv,
                        bv_full[:].rearrange("p (h c) -> p h c", h=H),
                        mybir.AluOpType.add)
                else:
                    if b % 2 == 0:
                        nc.vector.tensor_copy(vt[:, :, 0:HD], pv)
                    else:
                        nc.scalar.copy(vt[:, :, 0:HD], pv)
                nc.vector.memset(vt[:, :, HD:HD + 1], 1.0)
                v_sbs.append(vt)

            if stage <= 2:
                continue
            # per block: attention + out projection + gates
            ao_sbs = []
            for b in range(NBLK):
                n0 = b * 128
                vt = v_sbs[b]

                # scores S^T[m, n], all 8 heads, one 2-bank psum tile
                # head h = 2j + band; band-64 heads write psum bank 1 so
                # concurrently-running sub-array matmuls never share a bank
                pss_t = ps_s.tile([128, 2, SBN], F32, tag="s")
                pss = pss_t[:].rearrange("p g (j n) -> p g j n", j=4)
                for h in range(H):
                    p0 = (h % 2) * 64
                    nc.tensor.matmul(
                        pss[:, h % 2, h // 2, :],
                        qkt[p0:p0 + 64, 4 + h // 2, n0:n0 + 128],
                        qkt[p0:p0 + 64, h // 2, n0:n0 + 128],
                        start=True, stop=True)

                # exp of the two valid 64x64 quadrant sets (scale 1/sqrt(hd))
                attn = p_attn.tile([128, H, BLK], BF16, tag="attn")
                nc.gpsimd.memset(attn[0:64, :, 64:128], 0.0)
                nc.gpsimd.memset(attn[64:128, :, 0:64], 0.0)
                av = attn[:].rearrange("p (j g) n -> p g j n", g=2)
                nc.scalar.activation(
                    av[0:64, :, :, 0:64], pss[0:64, :, :, 0:64],
                    mybir.ActivationFunctionType.Exp, scale=0.125)
                nc.scalar.activation(
                    av[64:128, :, :, 64:128], pss[64:128, :, :, 64:128],
                    mybir.ActivationFunctionType.Exp, scale=0.125)

                if stage <= 3:
                    continue
                # ctx (+rowsum): [n, hd+1] per head, one 2-bank psum tile
                psc_t = ps_c.tile([128, 2, SBN], F32, tag="c")
                psc = psc_t[:, :, 0:4 * (HD + 1)].rearrange(
                    "p g (j c) -> p g j c", c=HD + 1)
                for h in range(H):
                    nc.tensor.matmul(
                        psc[:, h // 4, h % 4, :],
                        attn[:, h, :],
                        vt[:, h, :],
                        start=True, stop=True)
                rr = p_small.tile([128, 2, 4], F32, tag="rr")
                nc.vector.reciprocal(rr[:], psc[:, :, :, HD])
                ctx = p_ctx.tile([128, H, HD], BF16, tag="ctx")
                cv = ctx[:].rearrange("p (g j) c -> p g j c", g=2)
                nc.vector.tensor_tensor(
                    cv[:], psc[:, :, :, 0:HD],
                    rr[:, :, :, None].to_broadcast((128, 2, 4, HD)),
                    mybir.AluOpType.mult)

                if stage <= 4:
                    continue
                # ctx^T via PE transposes
                pst = ps_b.tile([128, D], BF16, tag="b")
                cflat = ctx[:].rearrange("p h c -> p (h c)")
                for ec in range(DC):
                    nc.tensor.transpose(
                        pst[:, ec * 128:(ec + 1) * 128],
                        cflat[:, ec * 128:(ec + 1) * 128],
                        ident_bf[:])
                ctxt = p_ctxt.tile([128, DC, 128], BF16, tag="ctxt")
                if b % 2 == 0:
                    nc.vector.tensor_copy(
                        ctxt[:].rearrange("p d n -> p (d n)"), pst[:])
                else:
                    nc.scalar.copy(
                        ctxt[:].rearrange("p d n -> p (d n)"), pst[:])

                if stage <= 5:
                    continue
                # out projection (natural [n, e]) + gate-logit column
                pso = ps_b.tile([128, D], F32, tag="b")
                for dc in range(DC):
                    nc.tensor.matmul(
                        pso[:], ctxt[:, dc, :], wo_sb[:, dc, :],
                        start=(dc == 0), stop=(dc == DC - 1))
                psg = ps_b.tile([128, 1], F32, tag="b")
                for dc in range(DC):
                    nc.tensor.matmul(
                        psg[:], ctxt[:, dc, :], weff_sb[:, dc, :],
                        start=(dc == 0), stop=(dc == DC - 1))
                ao = p_ao.tile([128, D], BF16, tag="ao")
                if has_bo:
                    nc.vector.tensor_tensor(
                        ao[:], pso[:], bo_full[:], mybir.AluOpType.add)
                else:
                    if b % 2 == 0:
                        nc.scalar.copy(ao[:], pso[:])
                    else:
                        nc.vector.tensor_copy(ao[:], pso[:])
                ao_sbs.append(ao)

                if stage <= 6:
                    continue
                # gates: sigmoid(x) = 0.5*tanh(x/2) + 0.5, written into the
                # two live half-columns of G_b
                tb = p_small.tile([128, 1], F32, tag="tb")
                nc.scalar.activation(
                    tb[:], psg[:],
                    mybir.ActivationFunctionType.Tanh,
                    bias=(gbh_full[:] if has_gb else 0.0), scale=0.5)
                nc.vector.tensor_scalar(
                    Gb[b][0:64, 2 * b:2 * b + 1], tb[0:64, :], 0.5, 0.5,
                    mybir.AluOpType.mult, mybir.AluOpType.add)
                nc.vector.tensor_scalar(
                    Gb[b][64:128, 2 * b + 1:2 * b + 2], tb[64:128, :],
                    0.5, 0.5,
                    mybir.AluOpType.mult, mybir.AluOpType.add)

            if stage <= 7:
                continue
            # readout: out[8 graphs, e] = sum_b G_b^T @ ao_b
            psro = ps_a.tile([8, SBN], F32, tag="a")
            for b in range(NBLK):
                nc.tensor.matmul(
                    psro[:], Gb[b][:, :], ao_sbs[b][:],
                    start=(b == 0), stop=(b == NBLK - 1))
            ro = p_small.tile([8, D], F32, tag="ro")
            if sb % 2 == 0:
                nc.vector.tensor_copy(ro[:], psro[:])
            else:
                nc.scalar.copy(ro[:], psro[:])
            nc.scalar.dma_start(out[sb * 8:(sb + 1) * 8, :], ro[:])

    import time as _time
    _t = _time.time()
    nc.compile()
    print(f"[kernel] bacc compile: {_time.time()-_t:.1f}s", flush=True)
    return nc


def kernel(x, batch, in_proj_w, in_proj_b, out_proj_w, out_proj_b,
           gate_w, gate_b):
    x = np.ascontiguousarray(np.asarray(x, dtype=np.float32))
    in_proj_w = np.asarray(in_proj_w, dtype=np.float32)
    in_proj_b = np.asarray(in_proj_b, dtype=np.float32)
    out_proj_w = np.asarray(out_proj_w, dtype=np.float32)
    out_proj_b = np.asarray(out_proj_b, dtype=np.float32)
    gate_w = np.asarray(gate_w, dtype=np.float32)
    gate_b = np.asarray(gate_b, dtype=np.float32)

    # host-side weight prep
    wqkT = in_proj_w[:2 * D].T                              # [512, 1024]
    wqk_h = np.ascontiguousarray(
        wqkT.reshape(DC, 128, 2 * D).transpose(1, 0, 2)).astype(ml_dtypes.bfloat16)
    wvT = in_proj_w[2 * D:].T                               # [512, 512]
    wv_h = np.ascontiguousarray(
        wvT.reshape(DC, 128, D).transpose(1, 0, 2)).astype(ml_dtypes.bfloat16)
    woT = out_proj_w.T                                      # [512, 512]
    wo_h = np.ascontiguousarray(
        woT.reshape(DC, 128, D).transpose(1, 0, 2)).astype(ml_dtypes.bfloat16)
    weff = (out_proj_w.T @ gate_w[0]).astype(np.float32)    # [512]
    weff_h = np.ascontiguousarray(
        weff.reshape(DC, 128, 1).transpose(1, 0, 2)).astype(ml_dtypes.bfloat16)

    bqk_np = in_proj_b[:2 * D]
    bv_np = in_proj_b[2 * D:]
    gb_eff = float(gate_b[0] + out_proj_b @ gate_w[0])
    has_bqk = bool(np.any(bqk_np))
    has_bv = bool(np.any(bv_np))
    has_bo = bool(np.any(out_proj_b))
    has_gb = gb_eff != 0.0

    import time as _time
    _t = _time.time()
    nc = _build(has_bqk, has_bv, has_bo, has_gb)
    print(f"[kernel] build total: {_time.time()-_t:.1f}s", flush=True)

    in_maps = []
    for c in range(N_CORES):
        m = {
            "xbf": np.ascontiguousarray(
                x[c * ROWS:(c + 1) * ROWS]).astype(ml_dtypes.bfloat16),
            "wqk": wqk_h, "wv": wv_h, "wo": wo_h, "weff": weff_h,
        }
        if has_bqk:
            m["bqk"] = np.ascontiguousarray(
                bqk_np.reshape(2 * DC, 128).T).astype(np.float32)
        if has_bv:
            m["bv"] = bv_np.reshape(1, D).astype(np.float32)
        if has_bo:
            m["bo"] = out_proj_b.reshape(1, D).astype(np.float32)
        if has_gb:
            m["gbh"] = np.array([[0.5 * gb_eff]], dtype=np.float32)
        in_maps.append(m)

    kernel.last_nc = nc
    kernel.last_in_maps = in_maps
    kernel.last_flags = (has_bqk, has_bv, has_bo, has_gb)

    res = run_bass_kernel_spmd(
        nc, in_maps, core_ids=list(range(N_CORES)), trace=TRACE)
    if TRACE:
        kernel.last_exec_time_ns = res.exec_time_ns
        kernel.last_results = res

    return np.concatenate([r["out"] for r in res.results], axis=0)


kernel.last_exec_time_ns = None
kernel.last_results = None
kernel.last_nc = None
kernel.last_in_maps = None


def _make_runner(nc, in_maps):
    """Build a repeat-callable PJRT runner for `nc` with device-resident
    inputs (mirrors bass2jax.run_bass_via_pjrt's multi-core path, minus
    output donation so buffers can be reused across timing iterations)."""
    import jax
    from jax.sharding import Mesh, PartitionSpec, NamedSharding
    from jax.experimental.shard_map import shard_map
    from concourse import bass2jax

    bass2jax.install_neuronx_cc_hook()
    n_cores = len(in_maps)

    partition_name = (
        nc.partition_id_tensor.name if nc.partition_id_tensor else None)
    in_names, out_names, out_avals, zero_outs = [], [], [], []
    for alloc in nc.m.functions[0].allocations:
        if not isinstance(alloc, mybir.MemoryLocationSet):
            continue
        name = alloc.memorylocations[0].name
        if alloc.kind == "ExternalInput":
            if name != partition_name:
                in_names.append(name)
        elif alloc.kind == "ExternalOutput":
            shape = tuple(alloc.tensor_shape)
            dtype = mybir.dt.np(alloc.dtype)
            out_avals.append(jax.core.ShapedArray(shape, dtype))
            out_names.append(name)
            zero_outs.append(np.zeros(shape, dtype))
    all_in_names = in_names + out_names
    if partition_name is not None:
        all_in_names = all_in_names + [partition_name]

    def _body(*args):
        operands = list(args)
        if partition_name is not None:
            operands.append(bass2jax.partition_id_tensor())
        outs = bass2jax._bass_exec_p.bind(
            *operands,
            out_avals=tuple(out_avals),
            in_names=tuple(all_in_names),
            out_names=tuple(out_names),
            lowering_input_output_aliases=(),
            sim_require_finite=True,
            sim_require_nnan=True,
            nc=nc,
        )
        return tuple(outs)

    devices = jax.devices()[:n_cores]
    mesh = Mesh(np.asarray(devices), ("core",))
    nsp = len(in_names) + len(out_names)
    sharded = jax.jit(
        shard_map(_body, mesh=mesh,
                  in_specs=(PartitionSpec("core"),) * nsp,
                  out_specs=(PartitionSpec("core"),) * len(out_names),
                  check_rep=False),
        keep_unused=True,
    )
    sharding = NamedSharding(mesh, PartitionSpec("core"))
    concat_in = [
        np.concatenate([np.asarray(in_maps[c][name]) for c in range(n_cores)], axis=0)
        for name in in_names
    ] + [np.zeros((n_cores * z.shape[0], *z.shape[1:]), z.dtype) for z in zero_outs]
    dev_in = [jax.device_put(a, sharding) for a in concat_in]

    def run_once(block=True):
        outs = sharded(*dev_in)
        if block:
            jax.block_until_ready(outs)
        return outs

    return run_once


def bench(iters=20, warmup=3):
    """Wall-clock timing is dispatch-dominated (~73 ms) and cannot resolve
    device time; kept for compatibility.  Reports x2-x1 marginal."""
    import time
    assert kernel.last_nc is not None, "call kernel() first"

    runner = _make_runner(kernel.last_nc, kernel.last_in_maps)
    nc_x2 = _build(*kernel.last_flags, variant=("x2",))
    runner_x2 = _make_runner(nc_x2, kernel.last_in_maps)

    def measure(run):
        for _ in range(warmup):
            run()
        ts = []
        for _ in range(iters):
            t0 = time.perf_counter()
            run()
            ts.append(time.perf_counter() - t0)
        ts.sort()
        return ts

    ts_k = measure(runner)
    ts_2 = measure(runner_x2)
    exec_ns = (ts_2[0] - ts_k[0]) * 1e9
    return exec_ns, ts_k[0] * 1e9, ts_2[0] * 1e9
